# revision 4
# baseline (speedup 1.0000x reference)
"""Distributed 2-layer GCN + mean-pool + linear classifier on 8 TRN2 NeuronCores.

Strategy (sharding hint: partition nodes + incident edges across cores):
  - Nodes are range-partitioned across the 8 cores (12500 real, padded 12544).
  - Each core owns the edges whose *destination* lies in its node range, so
    the scatter side of message passing is core-local.
  - Per GCN layer, each core computes g = D^-1/2 (h @ W) for its nodes on the
    TensorEngine, the g shards are AllGathered (split into 4 sub-collectives
    so gather tables stay int16-indexable), and each core pulls the source
    rows of its edges with dma_gather (one 512B descriptor per edge).
  - The per-destination reduction runs on the TensorEngine: tokens are
    pre-sorted by destination tile, a one-hot indicator is built on the
    VectorEngine (iota + is_equal against the per-token dst id) and
    matmul-accumulated into PSUM, one accumulation group per (quarter,
    dst-tile) run. Self-loop terms are folded in as the accumulator init.
  - deg^-1/2 scaling both pre-gather (src side) and post-aggregation (dst
    side) makes the edge weight dinv[s]*dinv[d] without any per-edge floats.
  - Mean-pool runs as one more indicator matmul into a [feat, graph] PSUM
    tile plus a count matmul, a [128,129] AllReduce, and the final linear
    is computed redundantly on every core.

All heavy traffic (feature gathers) is HBM-bandwidth-bound; host-side work is
restricted to integer index bucketing/sorting and degree/count histograms.
"""
import numpy as np

from concourse import bacc, bass, mybir, tile
from concourse.bass_utils import run_bass_kernel_spmd
from concourse.masks import make_identity

F32 = mybir.dt.float32
I16 = mybir.dt.int16

N_NODES = 100000
N_EDGES = 1600000
N_CORES = 8
F = 128
G = 128
OD = 16


def default_cfg():
    return dict(
        n=N_NODES, cores=N_CORES,
        nsh=N_NODES // N_CORES,      # real nodes per core
        nt=98,                       # node tiles per core (padded)
        q=4,                         # gather-table quarters
        ch_tiles=8,                 # token-tiles per gather chunk
    )


def _derived(cfg):
    npad = cfg["nt"] * 128
    qsh = npad // cfg["q"]
    tbl = cfg["cores"] * qsh
    assert npad % cfg["q"] == 0 and tbl <= 32768
    return npad, qsh, tbl


# ----------------------------------------------------------------- host prep

def host_prep(x, edge_index, batch, cfg):
    n, C, NSH, NT, Q = cfg["n"], cfg["cores"], cfg["nsh"], cfg["nt"], cfg["q"]
    NPAD, QSH, TBL = _derived(cfg)
    src = np.asarray(edge_index[0], dtype=np.int64)
    dst = np.asarray(edge_index[1], dtype=np.int64)
    batch = np.asarray(batch, dtype=np.int64)
    x = np.asarray(x, dtype=np.float32)

    deg = np.bincount(dst, minlength=n).astype(np.float32) + 1.0  # + self loop

    so = src // NSH
    r = src - so * NSH
    sq = r // QSH
    trow = so * QSH + (r - sq * QSH)

    down = dst // NSH
    dslot = dst - down * NSH
    dtile = dslot // 128
    drel = dslot - dtile * 128

    NQD = Q * NT
    seg_all = sq * NT + dtile
    counts = np.zeros((C, NQD), dtype=np.int64)
    for c in range(C):
        counts[c] = np.bincount(seg_all[down == c], minlength=NQD)

    caps_tiles = (counts.max(axis=0) + 127) // 128
    caps = caps_tiles * 128
    base = np.zeros(NQD, dtype=np.int64)
    np.cumsum(caps[:-1], out=base[1:])
    Ltot = int(caps.sum())
    q_start = np.zeros(Q + 1, dtype=np.int64)
    for q in range(Q):
        q_start[q + 1] = q_start[q] + int(caps[q * NT:(q + 1) * NT].sum())

    per_core = []
    for c in range(C):
        m = down == c
        tr_c, seg_c, rel_c = trow[m], seg_all[m], drel[m]
        order = np.lexsort((tr_c, seg_c))
        tr_c, seg_c, rel_c = tr_c[order], seg_c[order], rel_c[order]
        starts = np.zeros(NQD, dtype=np.int64)
        np.cumsum(counts[c][:-1], out=starts[1:])
        pos = base[seg_c] + (np.arange(len(seg_c)) - starts[seg_c])
        tr_pad = np.zeros(Ltot, dtype=np.int16)
        rel_pad = np.full(Ltot, -1.0, dtype=np.float32)
        tr_pad[pos] = tr_c.astype(np.int16)
        rel_pad[pos] = rel_c.astype(np.float32)

        idx_arrs, rel_arrs = [], []
        for q in range(Q):
            sl = slice(q_start[q], q_start[q + 1])
            Lq = int(q_start[q + 1] - q_start[q])
            iw = tr_pad[sl].reshape(Lq // 16, 16).T
            idx_arrs.append(np.tile(iw, (8, 1)).copy())              # [128, Lq/16] i16
            rel_arrs.append(rel_pad[sl].reshape(Lq // 128, 128).T.copy())  # [128, Lq/128] f32

        x_c = np.zeros((NPAD, F), dtype=np.float32)
        x_c[:NSH] = x[c * NSH:(c + 1) * NSH]
        deg_c = np.ones(NPAD, dtype=np.float32)
        deg_c[:NSH] = deg[c * NSH:(c + 1) * NSH]
        bat_c = np.full(NPAD, -1.0, dtype=np.float32)
        bat_c[:NSH] = batch[c * NSH:(c + 1) * NSH]
        per_core.append(dict(
            x=x_c,
            deg=deg_c.reshape(NT, 128).T.copy(),
            bat=bat_c.reshape(NT, 128).T.copy(),
            idx=idx_arrs, rel=rel_arrs,
        ))

    sched = []
    for q in range(Q):
        sched.append([(D, int(caps_tiles[q * NT + D])) for D in range(NT)
                      if caps_tiles[q * NT + D] > 0])
    Lqs = [int(q_start[q + 1] - q_start[q]) for q in range(Q)]
    return per_core, sched, Lqs


# ------------------------------------------------------------ program build

def build_program(cfg, sched, Lqs, b1, b2, bc):
    C, NT, Q, CHT = cfg["cores"], cfg["nt"], cfg["q"], cfg["ch_tiles"]
    NPAD, QSH, TBL = _derived(cfg)
    CH = CHT * 128
    rg = [list(range(C))]
    use_b1 = bool(np.any(b1)); use_b2 = bool(np.any(b2)); use_bc = bool(np.any(bc))

    nc = bacc.Bacc("TRN2", target_bir_lowering=False, debug=False,
                   num_devices=C)

    x_d = nc.dram_tensor("x", [NPAD, F], F32, kind="ExternalInput")
    deg_d = nc.dram_tensor("deg", [128, NT], F32, kind="ExternalInput")
    bat_d = nc.dram_tensor("bat", [128, NT], F32, kind="ExternalInput")
    w1_d = nc.dram_tensor("w1", [F, F], F32, kind="ExternalInput")
    w2_d = nc.dram_tensor("w2", [F, F], F32, kind="ExternalInput")
    wc_d = nc.dram_tensor("wc", [F, OD], F32, kind="ExternalInput")
    b1_d = nc.dram_tensor("b1b", [128, F], F32, kind="ExternalInput") if use_b1 else None
    b2_d = nc.dram_tensor("b2b", [128, F], F32, kind="ExternalInput") if use_b2 else None
    bc_d = nc.dram_tensor("bcb", [128, OD], F32, kind="ExternalInput") if use_bc else None
    idx_d = [nc.dram_tensor(f"idx{q}", [128, Lqs[q] // 16], I16,
                            kind="ExternalInput") for q in range(Q)]
    rel_d = [nc.dram_tensor(f"rel{q}", [128, Lqs[q] // 128], F32,
                            kind="ExternalInput") for q in range(Q)]
    out_d = nc.dram_tensor("out", [G, OD], F32, kind="ExternalOutput")

    gb_d = nc.dram_tensor("gbounce", [NPAD, F], F32)
    tbl_d = [nc.dram_tensor(f"tbl{q}", [TBL, F], F32, addr_space="Shared")
             for q in range(Q)]
    ar_in_d = nc.dram_tensor("ar_in", [128, F + 1], F32)
    ar_out_d = nc.dram_tensor("ar_out", [128, F + 1], F32, addr_space="Shared")

    with tile.TileContext(nc) as tc:
        with (
            tc.tile_pool(name="stat", bufs=1) as stat,
            tc.tile_pool(name="hA", bufs=NT) as poolA,
            tc.tile_pool(name="hB", bufs=NT) as poolB,
            tc.tile_pool(name="msg", bufs=3) as poolM,
            tc.tile_pool(name="ind", bufs=6) as poolI,
            tc.tile_pool(name="ldx", bufs=3) as poolL,
            tc.tile_pool(name="pt", bufs=1, space=bass.MemorySpace.PSUM) as pp_t,
            tc.tile_pool(name="pg", bufs=1, space=bass.MemorySpace.PSUM) as pp_g,
            tc.tile_pool(name="pa", bufs=4, space=bass.MemorySpace.PSUM) as pp_a,
            tc.tile_pool(name="pp", bufs=1, space=bass.MemorySpace.PSUM) as pp_p,
        ):
            # ------- static tiles
            ident = stat.tile([128, 128], F32, name="ident", tag="ident")
            make_identity(nc, ident[:])
            iota = stat.tile([128, 128], F32, name="iota", tag="iota")
            nc.gpsimd.iota(iota[:], pattern=[[1, 128]], base=0,
                           channel_multiplier=0,
                           allow_small_or_imprecise_dtypes=True)
            w1_s = stat.tile([F, F], F32, name="w1s", tag="w1s")
            nc.sync.dma_start(w1_s[:], w1_d[:])
            w2_s = stat.tile([F, F], F32, name="w2s", tag="w2s")
            nc.sync.dma_start(w2_s[:], w2_d[:])
            wc_s = stat.tile([F, OD], F32, name="wcs", tag="wcs")
            nc.sync.dma_start(wc_s[:], wc_d[:])
            deg_s = stat.tile([128, NT], F32, name="degs", tag="degs")
            nc.sync.dma_start(deg_s[:], deg_d[:])
            bat_s = stat.tile([128, NT], F32, name="bats", tag="bats")
            nc.sync.dma_start(bat_s[:], bat_d[:])
            sqd = stat.tile([128, NT], F32, name="sqd", tag="sqd")
            nc.scalar.sqrt(sqd[:], deg_s[:])
            dinv = stat.tile([128, NT], F32, name="dinv", tag="dinv")
            nc.vector.reciprocal(dinv[:], sqd[:])
            ones = stat.tile([128, 1], F32, name="ones", tag="ones")
            nc.vector.memset(ones[:], 1.0)
            bias_s = []
            for use, bd, shape in ((use_b1, b1_d, [128, F]),
                                   (use_b2, b2_d, [128, F]),
                                   (use_bc, bc_d, [128, OD])):
                if use:
                    t = stat.tile(shape, F32, name=f"bs{len(bias_s)}",
                                  tag=f"bs{len(bias_s)}")
                    nc.sync.dma_start(t[:], bd[:])
                    bias_s.append(t)
                else:
                    bias_s.append(None)

            hA = [poolA.tile([128, F], F32, name=f"hA{j}", tag="hA")
                  for j in range(NT)]
            hB = [poolB.tile([128, F], F32, name=f"hB{j}", tag="hB")
                  for j in range(NT)]

            def layer(li, w_s, bias_t):
                # --- g = dinv * (h @ w); h source: x from DRAM (L1) or hB (L2)
                for j in range(NT):
                    if li == 0:
                        xj = poolM.tile([128, F], F32, name=f"x{li}_{j}",
                                        tag="xin")
                        nc.sync.dma_start(xj[:], x_d[j * 128:(j + 1) * 128, :])
                    else:
                        xj = hB[j]
                    ptr = pp_t.tile([128, 128], F32, name=f"tp{li}_{j}",
                                    tag="ptr")
                    nc.tensor.transpose(ptr[:], xj[:], ident[:])
                    xT = poolM.tile([128, F], F32, name=f"xT{li}_{j}", tag="xT")
                    nc.scalar.copy(xT[:], ptr[:])
                    pg = pp_g.tile([128, F], F32, name=f"pg{li}_{j}", tag="pg")
                    nc.tensor.matmul(pg[:], xT[:], w_s[:])
                    nc.scalar.activation(hA[j][:], pg[:],
                                         mybir.ActivationFunctionType.Copy,
                                         scale=dinv[:, j:j + 1])
                    nc.sync.dma_start(gb_d[j * 128:(j + 1) * 128, :], hA[j][:])
                # --- allgather the scaled shard into the 4 tables
                for q in range(Q):
                    nc.gpsimd.collective_compute(
                        "AllGather", mybir.AluOpType.bypass,
                        replica_groups=rg,
                        ins=[gb_d[q * QSH:(q + 1) * QSH, :]],
                        outs=[tbl_d[q][:]],
                    )
                # --- gather + indicator-matmul aggregation, acc init = hA (self loop)
                for q in range(Q):
                    runs = sched[q]
                    flat = []  # (D, first, last) per token-tile
                    for (D, ntiles) in runs:
                        for t in range(ntiles):
                            flat.append((D, t == 0, t == ntiles - 1))
                    Lq = Lqs[q]
                    nchunks = (Lq + CH - 1) // CH
                    ti = 0
                    cur_psum = {}
                    for ci in range(nchunks):
                        t0 = ci * CHT
                        ntile = min(CHT, Lq // 128 - t0)
                        ntok = ntile * 128
                        idxt = poolL.tile([128, CH // 16], I16,
                                          name=f"ix{li}_{q}_{ci}", tag="idxt")
                        nc.sync.dma_start(
                            idxt[:, :ntok // 16],
                            idx_d[q][:, t0 * 8:t0 * 8 + ntok // 16])
                        relt = poolL.tile([128, CHT], F32,
                                          name=f"rl{li}_{q}_{ci}", tag="relt")
                        nc.sync.dma_start(relt[:, :ntile],
                                          rel_d[q][:, t0:t0 + ntile])
                        msg = poolM.tile([128, CHT, F], F32,
                                         name=f"mg{li}_{q}_{ci}", tag="msg")
                        nc.gpsimd.dma_gather(
                            msg[:, :ntile, :], tbl_d[q][:], idxt[:, :ntok // 16],
                            ntok, ntok, F)
                        for tj in range(ntile):
                            D, first, last = flat[ti]; ti += 1
                            ind = poolI.tile([128, 128], F32,
                                             name=f"in{li}_{q}_{ci}_{tj}",
                                             tag="ind")
                            nc.vector.tensor_scalar(
                                ind[:], iota[:], relt[:, tj:tj + 1],
                                None, mybir.AluOpType.is_equal)
                            if first:
                                pa = pp_a.tile([128, F], F32,
                                               name=f"pa{li}_{q}_{D}", tag="pa")
                                cur_psum[D] = pa
                            pa = cur_psum[D]
                            nc.tensor.matmul(pa[:], ind[:], msg[:, tj, :],
                                             start=first, stop=last)
                            if last:
                                nc.vector.tensor_tensor(
                                    out=hA[D][:], in0=hA[D][:], in1=pa[:],
                                    op=mybir.AluOpType.add)
                                del cur_psum[D]
                    assert ti == len(flat) == Lq // 128
                # --- finalize h = relu(dinv * acc (+ b))
                for j in range(NT):
                    if bias_t is None:
                        nc.scalar.activation(hB[j][:], hA[j][:],
                                             mybir.ActivationFunctionType.Relu,
                                             scale=dinv[:, j:j + 1])
                    else:
                        tmp = poolI.tile([128, F], F32, name=f"bt{li}_{j}",
                                         tag="ind")
                        nc.vector.tensor_scalar(tmp[:], hA[j][:],
                                                dinv[:, j:j + 1], None,
                                                mybir.AluOpType.mult)
                        nc.vector.tensor_tensor(out=tmp[:], in0=tmp[:],
                                                in1=bias_t[:],
                                                op=mybir.AluOpType.add)
                        nc.scalar.activation(hB[j][:], tmp[:],
                                             mybir.ActivationFunctionType.Relu)

            layer(0, w1_s, bias_s[0])
            layer(1, w2_s, bias_s[1])

            # ------- pooling: sums^T [feat, graph] and counts [graph, 1]
            ps = pp_p.tile([128, G], F32, name="psums", tag="psums")
            for j in range(NT):
                indg = poolI.tile([128, G], F32, name=f"ig{j}", tag="ind")
                nc.vector.tensor_scalar(indg[:], iota[:], bat_s[:, j:j + 1],
                                        None, mybir.AluOpType.is_equal)
                nc.tensor.matmul(ps[:], hB[j][:], indg[:],
                                 start=(j == 0), stop=(j == NT - 1))
            pn = pp_p.tile([128, 1], F32, name="pcnt", tag="pcnt")
            for j in range(NT):
                indg = poolI.tile([128, G], F32, name=f"ic{j}", tag="ind")
                nc.vector.tensor_scalar(indg[:], iota[:], bat_s[:, j:j + 1],
                                        None, mybir.AluOpType.is_equal)
                nc.tensor.matmul(pn[:], indg[:], ones[:],
                                 start=(j == 0), stop=(j == NT - 1))
            pack = stat.tile([128, F + 1], F32, name="pack", tag="pack")
            nc.scalar.copy(pack[:, 0:F], ps[:])
            nc.scalar.copy(pack[:, F:F + 1], pn[:])
            nc.sync.dma_start(ar_in_d[:], pack[:])
            nc.gpsimd.collective_compute(
                "AllReduce", mybir.AluOpType.add, replica_groups=rg,
                ins=[ar_in_d[:]], outs=[ar_out_d[:]])
            sums = stat.tile([128, F + 1], F32, name="sums", tag="sums")
            nc.sync.dma_start(sums[:], ar_out_d[:])
            cnt1 = stat.tile([128, 1], F32, name="cnt1", tag="cnt1")
            nc.vector.tensor_scalar_max(cnt1[:], sums[:, F:F + 1], 1.0)
            rcp = stat.tile([128, 1], F32, name="rcp", tag="rcp")
            nc.vector.reciprocal(rcp[:], cnt1[:])
            po = pp_g.tile([128, OD], F32, name="po", tag="pg")
            nc.tensor.matmul(po[:], sums[:, 0:F], wc_s[:])
            osb = stat.tile([128, OD], F32, name="osb", tag="osb")
            nc.scalar.activation(osb[:], po[:],
                                 mybir.ActivationFunctionType.Copy,
                                 scale=rcp[:])
            if bias_s[2] is not None:
                nc.vector.tensor_tensor(out=osb[:], in0=osb[:],
                                        in1=bias_s[2][:],
                                        op=mybir.AluOpType.add)
            nc.sync.dma_start(out_d[:], osb[:])

    nc.compile()
    return nc


# ------------------------------------------------------------------- driver

def run(inputs, cfg, trace=False):
    x = np.asarray(inputs["x"], dtype=np.float32)
    edge_index = np.asarray(inputs["edge_index"])
    batch = np.asarray(inputs["batch"])
    w1 = np.asarray(inputs["w1"], dtype=np.float32)
    b1 = np.asarray(inputs["b1"], dtype=np.float32)
    w2 = np.asarray(inputs["w2"], dtype=np.float32)
    b2 = np.asarray(inputs["b2"], dtype=np.float32)
    wc = np.asarray(inputs["wc"], dtype=np.float32)
    bc = np.asarray(inputs["bc"], dtype=np.float32)

    per_core, sched, Lqs = host_prep(x, edge_index, batch, cfg)
    nc = build_program(cfg, sched, Lqs, b1, b2, bc)

    in_maps = []
    for c in range(cfg["cores"]):
        pc = per_core[c]
        m = dict(x=pc["x"], deg=pc["deg"], bat=pc["bat"],
                 w1=w1, w2=w2, wc=wc)
        for q in range(cfg["q"]):
            m[f"idx{q}"] = pc["idx"][q]
            m[f"rel{q}"] = pc["rel"][q]
        if np.any(b1):
            m["b1b"] = np.tile(b1[None, :], (128, 1)).astype(np.float32)
        if np.any(b2):
            m["b2b"] = np.tile(b2[None, :], (128, 1)).astype(np.float32)
        if np.any(bc):
            m["bcb"] = np.tile(bc[None, :], (128, 1)).astype(np.float32)
        in_maps.append(m)

    res = run_bass_kernel_spmd(nc, in_maps, list(range(cfg["cores"])),
                               trace=trace)
    out = np.asarray(res.results[0]["out"], dtype=np.float32)
    return out, res


def kernel(**inputs) -> np.ndarray:
    out, _ = run(inputs, default_cfg(), trace=False)
    return out
